# revision 15
# baseline (speedup 1.0000x reference)
"""Fused Conv1d(up=2) + FIR resample + bias for TRN2, data-parallel over batch.

Math (polyphase decomposition, verified against the reference):
  with kf = (1,3,1)/5 * 2 = (0.4, 1.2, 0.4):
    out[2i]   = x[i-1]@A + x[i]@B + b
    out[2i+1] = x[i-1]@C + x[i]@D + x[i+1]@E + b
    A = 1.2*w0 + 0.4*w1    B = 0.4*w1 + 1.2*w2    C = 0.4*w0
    D = 0.4*w0 + 1.2*w1 + 0.4*w2                  E = 0.4*w2

Layout: channels on partitions, tokens on the free dim. PSUM tiles are
[outC-chunk(128), 512 tokens] = exactly one bank; 4 tags x 2 bufs fill all
8 banks. Everything moves in bf16 (host converts for free): ~2.9 MiB loads
+ 4 MiB stores per core, far under the PE roofline (81,920 matmul rows =
34.1 us at 2.4 GHz), so the kernel is PE-bound.

Startup is the battle: the NEFF preamble runs ~7.5 us, then the HAM
activity monitor requires ~2.7 us of CONTINUOUS PE activity before granting
full clock (an idle gap resets the counter). Junk matmuls start the
activity the moment the Tensor engine frees, sized to still be running when
the first real matmul's operands land. Loads are ordered so the first
matmuls' dependencies are minimal (weights m-chunk-major in 4 DMAs, block-0
x split per K-chunk, kc-major tap order). Loads go on the sync HWDGE
queue, stores split across the scalar AND sync queues (the only two
hardware queues): scalar stores the act-engine bands, sync the DVE bands,
so each store waits only on its own producer.

Bias + bf16 cast happen while draining PSUM: M-chunk 0 via scalar-engine
activation(Identity, per-partition bias), M-chunk 1 via DVE tensor_scalar,
in parallel, both far under the PE time. Output goes to DRAM as
[128, block, 4, 512] bf16 (2 KB contiguous per partition per store) and the
host reassembles [8192, 256] fp32.
"""

import ml_dtypes
import numpy as np

import concourse.bass as bass
import concourse.mybir as mybir
import concourse.tile as tile
from concourse import bacc
from concourse.bass_utils import run_bass_kernel_spmd

N_CORES = 8
H = 4096  # tokens per core
C = 256  # channels
P = 128  # SBUF partitions
BLK = 512  # output tokens per block = one PSUM bank of fp32
NBLK = H // BLK  # 8 blocks
BF16 = ml_dtypes.bfloat16

_NC_CACHE = None


def _build_nc():
    f32 = mybir.dt.float32
    bf16 = mybir.dt.bfloat16
    AF = mybir.ActivationFunctionType
    OP = mybir.AluOpType
    nc = bacc.Bacc(
        "TRN2",
        target_bir_lowering=False,
        debug=False,
        enable_asserts=False,
        num_devices=N_CORES,
    )
    xT = nc.dram_tensor("xT", [C, H], bf16, kind="ExternalInput").ap()
    # lhsT weights packed per m-chunk, per-partition contiguous:
    # col block (2*mat+kc) holds lhsT[inC kc*128+p, outC m*128+c]
    wAB0 = nc.dram_tensor("wAB0", [P, 4 * P + 2], bf16, kind="ExternalInput").ap()
    wAB1 = nc.dram_tensor("wAB1", [P, 4 * P], bf16, kind="ExternalInput").ap()
    wCDE0 = nc.dram_tensor("wCDE0", [P, 6 * P], bf16, kind="ExternalInput").ap()
    wCDE1 = nc.dram_tensor("wCDE1", [P, 6 * P], bf16, kind="ExternalInput").ap()
        # out[p, b, s, j]: band s in (even-m0, odd-m0, even-m1, odd-m1),
    # value = out token b*512+j (parity per band), channel m*128+p
    out = nc.dram_tensor("out", [P, NBLK * 4 * BLK], bf16, kind="ExternalOutput").ap()

    with tile.TileContext(nc) as tc:
        with (
            tc.tile_pool(name="consts", bufs=1) as consts,
            tc.tile_pool(name="spool", bufs=3) as spool,
            tc.tile_pool(name="psum", bufs=2, space="PSUM") as psum,
        ):
            xT_v = xT.rearrange("(c p) h -> p c h", p=P)  # [128, 2, H]
            out_v = out.rearrange("p (b s j) -> p b s j", b=NBLK, s=4)

            # Junk-matmul warmup: start PE activity the moment the Tensor
            # engine frees so the HAM window is tripped ~2.7us later, just
            # as the real stream takes over.
            junkL = consts.tile([P, P], bf16, tag="junkL")
            junkR = consts.tile([P, BLK], bf16, tag="junkR")
            nc.vector.memset(junkL[:], 0.0)
            nc.vector.memset(junkR[:], 0.0)

            # Loads (sync queue) in dependency-priority order.
            wab = {}
            wcde = {}
            wab[0] = consts.tile([P, 4 * P + 2], bf16, tag="wab0", name="wab0")
            nc.scalar.dma_start(wab[0][:], wAB0)

            # x tiles: (lo token, cols, blocks covered); tile col j = x[lo+j]
            # block 0 is column-split in halves with a tiny first tile so the
            # first real matmul is ready before the junk chain ends.
            XTILES = [(-1, 260, ()), (255, 260, ()), (511, 1028, (1, 2)),
                      (1535, 1028, (3, 4)), (2559, 1028, (5, 6)),
                      (3583, 515, (7,))]
            xt = {}  # block -> (tile, col offset of x[B-1])
            xtiles = []

            def load_x(ti):
                lo, ncols, blks = XTILES[ti]
                t = consts.tile([P, 2, ncols], bf16, tag=f"xt{ti}", name=f"xt{ti}")
                hi = lo + ncols  # exclusive
                src_lo, src_hi = max(lo, 0), min(hi, H)
                d0 = src_lo - lo
                if lo < 0:
                    nc.vector.memset(t[:, :, 0:d0], 0.0)
                if hi > H:
                    nc.vector.memset(t[:, :, d0 + (src_hi - src_lo) :], 0.0)
                nc.sync.dma_start(
                    t[:, :, d0 : d0 + (src_hi - src_lo)], xT_v[:, :, src_lo:src_hi]
                )
                for b in blks:
                    xt[b] = (t, b * BLK - 1 - lo)
                xtiles.append(t)

            load_x(0)
            load_x(1)
            wab[1] = consts.tile([P, 4 * P], bf16, tag="wab1", name="wab1")
            nc.scalar.dma_start(wab[1][:], wAB1)
            wcde[0] = consts.tile([P, 6 * P], bf16, tag="wcde0", name="wcde0")
            nc.scalar.dma_start(wcde[0][:], wCDE0)
            wcde[1] = consts.tile([P, 6 * P], bf16, tag="wcde1", name="wcde1")
            nc.scalar.dma_start(wcde[1][:], wCDE1)

            bias = consts.tile([P, 2], f32, tag="bias")
            nc.vector.tensor_copy(bias[:], wab[0][:, 4 * P : 4 * P + 2])

            for _ in range(7):
                psj = psum.tile([P, BLK], f32, tag="psE0")
                nc.tensor.matmul(psj[:], junkL[:], junkR[:], start=True, stop=True)

            for ti in range(2, len(XTILES)):
                load_x(ti)

            def wslice(tile_ap, mat, kc):
                a = 2 * mat + kc
                return tile_ap[:, a * P : (a + 1) * P]

            # (matrix index within its tile, x-column offset)
            E_TAPS = ((0, 0), (1, 1))  # A@x[i-1], B@x[i]
            O_TAPS = ((0, 0), (1, 1), (2, 2))  # C@x[i-1], D@x[i], E@x[i+1]

            for b in range(NBLK):
                ps = {}
                if b == 0:
                    # column-split halves: (x tile, psum col offset, width)
                    parts = [(xtiles[0], 0, 256), (xtiles[1], 256, 256)]
                else:
                    parts = [(xt[b][0], 0, BLK)]
                    xoff = xt[b][1]
                for m in range(2):
                    ps[m] = psum.tile([P, BLK], f32, tag=f"psE{m}", name=f"psE{m}")
                for m in (0, 1) if b < NBLK - 1 else (1, 0):
                    ps[2 + m] = psum.tile([P, BLK], f32, tag=f"psO{m}", name=f"psO{m}")
                for x, c0, w_ in parts:
                    xo = -c0 if b == 0 else xoff
                    for m in range(2):
                        for i, (kc, (mat, d)) in enumerate(
                            [(kc, t) for kc in range(2) for t in E_TAPS]
                        ):
                            nc.tensor.matmul(
                                ps[m][:, c0 : c0 + w_], wslice(wab[m], mat, kc),
                                x[:, kc, xo + c0 + d : xo + c0 + d + w_],
                                start=(i == 0), stop=(i == 3),
                            )
                    for m in range(2):
                        for i, (kc, (mat, d)) in enumerate(
                            [(kc, t) for kc in range(2) for t in O_TAPS]
                        ):
                            nc.tensor.matmul(
                                ps[2 + m][:, c0 : c0 + w_], wslice(wcde[m], mat, kc),
                                x[:, kc, xo + c0 + d : xo + c0 + d + w_],
                                start=(i == 0), stop=(i == 5),
                            )

                stage = spool.tile([P, 4, BLK], bf16, tag="stage")
                # bands: 0=even-m0, 1=odd-m0 (scalar), 2=even-m1, 3=odd-m1 (DVE)
                if b < NBLK - 1:
                    nc.scalar.activation(stage[:, 0, :], ps[0][:], AF.Identity,
                                         bias=bias[:, 0:1], scale=1.0)
                    nc.scalar.activation(stage[:, 1, :], ps[2][:], AF.Identity,
                                         bias=bias[:, 0:1], scale=1.0)
                    nc.vector.tensor_scalar(stage[:, 2, :], ps[1][:], bias[:, 1:2], None, OP.add)
                    nc.vector.tensor_scalar(stage[:, 3, :], ps[3][:], bias[:, 1:2], None, OP.add)
                    nc.scalar.dma_start(out_v[:, b, 0:2, :], stage[:, 0:2, :])
                    nc.sync.dma_start(out_v[:, b, 2:4, :], stage[:, 2:4, :])
                else:
                    # last block: store each band the moment its producer
                    # finishes so the drain after the final matmul is short
                    nc.scalar.activation(stage[:, 0, :], ps[0][:], AF.Identity,
                                         bias=bias[:, 0:1], scale=1.0)
                    nc.scalar.dma_start(out_v[:, b, 0:1, :], stage[:, 0:1, :])
                    nc.vector.tensor_scalar(stage[:, 2, :], ps[1][:], bias[:, 1:2], None, OP.add)
                    nc.sync.dma_start(out_v[:, b, 2:3, :], stage[:, 2:3, :])
                    nc.vector.tensor_scalar(stage[:, 3, :], ps[3][:], bias[:, 1:2], None, OP.add)
                    nc.sync.dma_start(out_v[:, b, 3:4, :], stage[:, 3:4, :])
                    nc.scalar.activation(stage[:, 1, :], ps[2][:], AF.Identity,
                                         bias=bias[:, 0:1], scale=1.0)
                    nc.scalar.dma_start(out_v[:, b, 1:2, :], stage[:, 1:2, :])

    nc.compile()
    return nc


def _get_nc():
    global _NC_CACHE
    if _NC_CACHE is None:
        _NC_CACHE = _build_nc()
    return _NC_CACHE


def _pack_w_m(mats, m):
    # lhsT blocks for m-chunk m: [128, len(mats)*2*128], col block (2*mat+kc)
    blocks = [
        mat[kc * P : (kc + 1) * P, m * P : (m + 1) * P]
        for mat in mats
        for kc in range(2)
    ]
    return np.ascontiguousarray(
        np.stack(blocks, axis=1).reshape(P, len(blocks) * P)
    ).astype(BF16)


def _prep_in_maps(x, w, b):
    x = np.asarray(x, np.float32)  # [8, 4096, 256]
    w = np.asarray(w, np.float32)  # [3, 256, 256] = [K, inC, outC]
    b = np.asarray(b, np.float32)  # [256]
    kf0, kf1 = np.float32(0.4), np.float32(1.2)
    w0, w1, w2 = w[0], w[1], w[2]
    A = kf1 * w0 + kf0 * w1
    Bm = kf0 * w1 + kf1 * w2
    Cm = kf0 * w0
    D = kf0 * w0 + kf1 * w1 + kf0 * w2
    E = kf0 * w2
    wab0 = np.concatenate(
        [_pack_w_m([A, Bm], 0).astype(np.float32),
         b.reshape(2, P).T.astype(np.float32)], axis=1)
    shared = {
        "wAB0": np.ascontiguousarray(wab0).astype(BF16),
        "wAB1": _pack_w_m([A, Bm], 1),
        "wCDE0": _pack_w_m([Cm, D, E], 0),
        "wCDE1": _pack_w_m([Cm, D, E], 1),
    }
    return [
        {"xT": np.ascontiguousarray(x[i].T).astype(BF16), **shared}
        for i in range(N_CORES)
    ]


def kernel(x, w, b):
    nc = _get_nc()
    in_maps = _prep_in_maps(x, w, b)
    res = run_bass_kernel_spmd(nc, in_maps, list(range(N_CORES)))
    out = np.empty((N_CORES, 2 * H, C), np.float32)
    for i in range(N_CORES):
        dev = np.asarray(res.results[i]["out"]).astype(np.float32)
        # dev[p, b, s, j]: s = 2*m + parity ; out row 2*(512b+j)+parity
        arr = dev.reshape(P, NBLK, 2, 2, BLK)  # [p, b, m, parity, j]
        out[i] = arr.transpose(1, 4, 3, 2, 0).reshape(2 * H, C)
    return out
